# revision 3
# baseline (speedup 1.0000x reference)
"""Trainium2 Bass kernel for nn_ColonyCBF (gnn_message_passing).

Computation (per row b of B=2^21):
    x_flat = concat(x_local[b], x_all[b, 1:7, :])            # 28 features
    h1 = relu(x_flat @ W1 + b1)                              # 64
    h2 = relu(h1 @ W2 + b2)                                  # 32
    out = 0.3 - softmax(|rw|) . x_local[b] + 0.1*(h2 @ W3 + b3)

Strategy: pure data-parallel over 8 NeuronCores.  On the host the batch is
packed into a transposed, 4-way "pack" layout (feature-on-partition) in bf16
so the device kernel is nothing but streaming matmuls with block-diagonal
weights, two PSUM->SBUF relu passes, and a fused risk/L3 accumulation:

  xt [128, BC/4] bf16 : partition strip 32r+f = feature f of batch quarter r
  L1: two [128,128] block-diag matmuls -> H1 packs 0,1 / 2,3 (PSUM A, B)
  relu1 (ACT, bias) -> bf16 H1 tiles
  L2: two col-tiled [128,64] matmuls into one PSUM bank -> H2 (4 packs)
  relu2 (DVE tensor_scalar add+max) -> bf16 H2 tile
  risk: [128,4] matmul vs xt (start=True) accumulated with
  L3:   [128,4] matmul vs H2 (start=False) -> psum G [4, N]
  final (ACT/DVE, bias 0.3+0.1*b3) -> staging, grouped stores to DRAM
"""

import sys
import os
import numpy as np
import ml_dtypes

sys.path.insert(0, "/opt/trn_rl_repo")

BF16 = ml_dtypes.bfloat16

B = 2_097_152
N_CORES = 8
BC = B // N_CORES            # rows per core
QUARTER = BC // 4            # columns of the packed layout
N = 512                      # columns (batch rows / 4) per chunk
N_CHUNKS = QUARTER // N      # 128
XGROUP = 8                   # chunks per input DMA
SGROUP = 16                  # chunks per staging tile / store group

_BUILD_CACHE = {}


def _build(repeat=1):
    key = repeat
    if key in _BUILD_CACHE:
        return _BUILD_CACHE[key]
    import concourse.mybir as mybir
    import concourse.tile as tile
    from concourse import bacc
    from concourse.alu_op_type import AluOpType
    from contextlib import ExitStack

    dt = mybir.dt
    AF = mybir.ActivationFunctionType

    nc = bacc.Bacc("TRN2", target_bir_lowering=False, debug=False,
                   num_devices=N_CORES)
    xt_d = nc.dram_tensor("xt", [128, QUARTER], dt.bfloat16, kind="ExternalInput").ap()
    w1a_d = nc.dram_tensor("w1a", [128, 128], dt.bfloat16, kind="ExternalInput").ap()
    w1b_d = nc.dram_tensor("w1b", [128, 128], dt.bfloat16, kind="ExternalInput").ap()
    w2_d = nc.dram_tensor("w2", [128, 64], dt.bfloat16, kind="ExternalInput").ap()
    w3_d = nc.dram_tensor("w3", [128, 4], dt.bfloat16, kind="ExternalInput").ap()
    wr_d = nc.dram_tensor("wr", [128, 4], dt.bfloat16, kind="ExternalInput").ap()
    b1r_d = nc.dram_tensor("b1r", [128, 1], dt.float32, kind="ExternalInput").ap()
    b2r_d = nc.dram_tensor("b2r", [128, 1], dt.float32, kind="ExternalInput").ap()
    bf_d = nc.dram_tensor("bfin", [128, 1], dt.float32, kind="ExternalInput").ap()
    y_d = nc.dram_tensor("y", [4, QUARTER], dt.float32, kind="ExternalOutput").ap()

    with tile.TileContext(nc) as tc, ExitStack() as ctx:
        consts = ctx.enter_context(tc.tile_pool(name="consts", bufs=1))
        xpool = ctx.enter_context(tc.tile_pool(name="x", bufs=3))
        h1pool = ctx.enter_context(tc.tile_pool(name="h1", bufs=4))
        h2pool = ctx.enter_context(tc.tile_pool(name="h2", bufs=3))
        stpool = ctx.enter_context(tc.tile_pool(name="stage", bufs=2))
        psA = ctx.enter_context(tc.tile_pool(name="psA", bufs=2, space="PSUM"))
        psB = ctx.enter_context(tc.tile_pool(name="psB", bufs=2, space="PSUM"))
        psE = ctx.enter_context(tc.tile_pool(name="psE", bufs=2, space="PSUM"))
        psG = ctx.enter_context(tc.tile_pool(name="psG", bufs=2, space="PSUM"))

        def cl(dram, shape, dtype):
            t = consts.tile(shape, dtype, tag=dram.tensor.name)
            nc.sync.dma_start(out=t, in_=dram)
            return t

        s_w1a = cl(w1a_d, [128, 128], dt.bfloat16)
        s_w1b = cl(w1b_d, [128, 128], dt.bfloat16)
        s_w2 = cl(w2_d, [128, 64], dt.bfloat16)
        s_w3 = cl(w3_d, [128, 4], dt.bfloat16)
        s_wr = cl(wr_d, [128, 4], dt.bfloat16)
        s_b1r = cl(b1r_d, [128, 1], dt.float32)
        s_b2r = cl(b2r_d, [128, 1], dt.float32)
        s_bf = cl(bf_d, [128, 1], dt.float32)

        # output view: quarter r, stage s, col-block z, strip c, col n
        y_v = y_d.rearrange("r (S z c n) -> S c r z n", z=4, c=4, n=N)

        def body():
            for s in range(N_CHUNKS // SGROUP):
                stage = stpool.tile([128, 4 * N], dt.float32)
                for jq in range(SGROUP):
                    j = s * SGROUP + jq
                    if j % XGROUP == 0:
                        xbig = xpool.tile([128, XGROUP * N], dt.bfloat16, tag="xbig")
                    xtile = xbig[:, (j % XGROUP) * N:(j % XGROUP + 1) * N]
                    if j % XGROUP == 0:
                        nc.sync.dma_start(
                            out=xbig,
                            in_=xt_d[:, (j // XGROUP) * XGROUP * N:
                                     (j // XGROUP + 1) * XGROUP * N])
                    pA = psA.tile([128, N], dt.float32)
                    nc.tensor.matmul(pA, s_w1a, xtile, start=True, stop=True)
                    pB = psB.tile([128, N], dt.float32)
                    nc.tensor.matmul(pB, s_w1b, xtile, start=True, stop=True)
                    pG = psG.tile([4, N], dt.float32)
                    nc.tensor.matmul(pG, s_wr, xtile, start=True, stop=False,
                                     skip_group_check=True)
                    h1a = h1pool.tile([128, N], dt.bfloat16, tag="h1")
                    nc.scalar.activation(h1a, pA, AF.Relu, bias=s_b1r, scale=1.0)
                    h1b = h1pool.tile([128, N], dt.bfloat16, tag="h1")
                    if jq % 4 == 0:
                        # keep ACT/DVE balanced: 1-in-4 relu1b goes to ACT
                        nc.scalar.activation(h1b, pB, AF.Relu, bias=s_b1r, scale=1.0)
                    else:
                        nc.vector.tensor_scalar(out=h1b, in0=pB, scalar1=s_b1r,
                                                scalar2=0.0, op0=AluOpType.add,
                                                op1=AluOpType.max)
                    pE = psE.tile([128, N], dt.float32)
                    nc.tensor.matmul(pE[0:64, :], s_w2, h1a, start=True, stop=True,
                                     tile_position=(0, 0))
                    nc.tensor.matmul(pE[64:128, :], s_w2, h1b, start=True, stop=True,
                                     tile_position=(0, 64))
                    h2t = h2pool.tile([128, N], dt.bfloat16)
                    nc.vector.tensor_scalar(out=h2t, in0=pE, scalar1=s_b2r,
                                            scalar2=0.0, op0=AluOpType.add,
                                            op1=AluOpType.max)
                    nc.tensor.matmul(pG, s_w3, h2t, start=False, stop=True,
                                     skip_group_check=True)
                    # chunk j -> stage partitions 32c:32c+4, col block z
                    c = jq % 4
                    z = jq // 4
                    out_ap = stage[32 * c:32 * c + 4, z * N:(z + 1) * N]
                    nc.scalar.activation(out_ap, pG, AF.Identity,
                                         bias=s_bf[0:4], scale=1.0)
                for c in range(4):
                    nc.sync.dma_start(out=y_v[s, c], in_=stage[32 * c:32 * c + 4, :])

        if repeat > 1:
            with tc.For_i(0, repeat, 1):
                body()
        else:
            body()

    nc.compile()
    _BUILD_CACHE[key] = nc
    return nc


def _prep_inputs(x_local, x_all, W1, b1, W2, b2, W3, b3, risk_weights):
    xf = np.empty((B, 28), np.float32)
    xf[:, :4] = x_local
    xf[:, 4:] = x_all[:, 1:7, :].reshape(B, 24)
    xb = xf.astype(BF16)
    X = xb.reshape(N_CORES, 4, QUARTER, 28)

    w1a = np.zeros((128, 128), BF16)
    w1a[0:28, 0:64] = W1
    w1a[32:60, 64:128] = W1
    w1b = np.zeros((128, 128), BF16)
    w1b[64:92, 0:64] = W1
    w1b[96:124, 64:128] = W1
    w2m = np.zeros((128, 64), BF16)
    w2m[0:64, 0:32] = W2
    w2m[64:128, 32:64] = W2
    w3m = np.zeros((128, 4), BF16)
    for r in range(4):
        w3m[32 * r:32 * r + 32, r] = 0.1 * W3[:, 0]
    a = np.abs(risk_weights.astype(np.float32))
    e = np.exp(a - a.max())
    wsm = e / e.sum()
    wrm = np.zeros((128, 4), BF16)
    for r in range(4):
        wrm[32 * r:32 * r + 4, r] = -wsm
    b1r = np.tile(np.asarray(b1, np.float32), 2).reshape(128, 1)
    b2r = np.tile(np.asarray(b2, np.float32), 4).reshape(128, 1)
    bfin = np.full((128, 1), 0.3 + 0.1 * float(b3[0]), np.float32)

    consts = dict(w1a=w1a, w1b=w1b, w2=w2m, w3=w3m, wr=wrm,
                  b1r=b1r, b2r=b2r, bfin=bfin)
    in_maps = []
    for c in range(N_CORES):
        xt = np.zeros((4, 32, QUARTER), BF16)
        xt[:, :28, :] = X[c].transpose(0, 2, 1)
        in_maps.append(dict(xt=xt.reshape(128, QUARTER), **consts))
    return in_maps


def run(in_maps, repeat=1):
    from concourse.bass_utils import run_bass_kernel_spmd
    nc = _build(repeat)
    return run_bass_kernel_spmd(nc, in_maps, core_ids=list(range(N_CORES)))


def kernel(x_local, x_all, W1, b1, W2, b2, W3, b3, risk_weights):
    x_local = np.asarray(x_local)
    x_all = np.asarray(x_all)
    in_maps = _prep_inputs(x_local, x_all, np.asarray(W1), np.asarray(b1),
                           np.asarray(W2), np.asarray(b2), np.asarray(W3),
                           np.asarray(b3), np.asarray(risk_weights))
    res = run(in_maps)
    out = np.empty(B, np.float32)
    for c in range(N_CORES):
        out[c * BC:(c + 1) * BC] = np.asarray(res.results[c]["y"],
                                              np.float32).reshape(-1)
    return out


# revision 4
# speedup vs baseline: 1.0001x; 1.0001x over previous
"""Trainium2 Bass kernel for nn_ColonyCBF (gnn_message_passing).

Computation (per row b of B=2^21):
    x_flat = concat(x_local[b], x_all[b, 1:7, :])            # 28 features
    h1 = relu(x_flat @ W1 + b1)                              # 64
    h2 = relu(h1 @ W2 + b2)                                  # 32
    out = 0.3 - softmax(|rw|) . x_local[b] + 0.1*(h2 @ W3 + b3)

Strategy: pure data-parallel over 8 NeuronCores.  On the host the batch is
packed into a transposed, 4-way "pack" layout (feature-on-partition) in bf16
so the device kernel is nothing but streaming matmuls with block-diagonal
weights and PSUM->SBUF relu passes:

  xt [128, BC/4] bf16 : partition strip 32r+f = feature f of batch quarter r
  L1:   8 concurrent 32x32 tile_position matmuls -> H1 (PSUM A, B)
  risk: 4 concurrent 32x32 tiles on the free diagonal cells, writing
        pG partitions {0,32,64,96} (pack r -> partition 32*POS[r])
  relu1 (ACT/DVE, bias) -> bf16 H1 tiles
  L2:   two col-tiled [128,64] matmuls into one PSUM bank -> H2 (4 packs)
  relu2 (DVE tensor_scalar add+max) -> bf16 H2 tile
  L3:   wide-M [128,97] matmul vs H2 accumulated onto risk (start=False)
  final (ACT, bias 0.3+0.1*b3) [97,N] -> staging rows {0,32,64,96}
  store: per strip c, one contiguous [1, 16N] DMA into output quarter INV[c]
"""

import sys
import numpy as np
import ml_dtypes

sys.path.insert(0, "/opt/trn_rl_repo")

BF16 = ml_dtypes.bfloat16

B = 2_097_152
N_CORES = 8
BC = B // N_CORES            # rows per core
QUARTER = BC // 4            # columns of the packed layout
N = 512                      # columns (batch rows / 4) per chunk
N_CHUNKS = QUARTER // N      # 128
XGROUP = 8                   # chunks per input DMA
SGROUP = 16                  # chunks per staging tile / store group

# risk/L3 output placement: pack r -> free PE diagonal cell (32r, 32*POS[r])
# (cells (r, 0..3) with L1 occupying (0,{0,1}),(1,{2,3}),(2,{0,1}),(3,{2,3}))
POS = (2, 0, 3, 1)
# stage strip c holds pack INV[c]
INV = (1, 3, 0, 2)

_BUILD_CACHE = {}


def _build(repeat=1):
    key = repeat
    if key in _BUILD_CACHE:
        return _BUILD_CACHE[key]
    import concourse.mybir as mybir
    import concourse.tile as tile
    from concourse import bacc
    from concourse.alu_op_type import AluOpType
    from contextlib import ExitStack

    dt = mybir.dt
    AF = mybir.ActivationFunctionType

    nc = bacc.Bacc("TRN2", target_bir_lowering=False, debug=False,
                   num_devices=N_CORES)
    xt_d = nc.dram_tensor("xt", [128, QUARTER], dt.bfloat16, kind="ExternalInput").ap()
    w1a_d = nc.dram_tensor("w1a", [128, 128], dt.bfloat16, kind="ExternalInput").ap()
    w1b_d = nc.dram_tensor("w1b", [128, 128], dt.bfloat16, kind="ExternalInput").ap()
    w2_d = nc.dram_tensor("w2", [128, 64], dt.bfloat16, kind="ExternalInput").ap()
    w3_d = nc.dram_tensor("w3", [128, 97], dt.bfloat16, kind="ExternalInput").ap()
    wr_d = nc.dram_tensor("wr", [128, 4], dt.bfloat16, kind="ExternalInput").ap()
    b1r_d = nc.dram_tensor("b1r", [128, 1], dt.float32, kind="ExternalInput").ap()
    b2r_d = nc.dram_tensor("b2r", [128, 1], dt.float32, kind="ExternalInput").ap()
    bf_d = nc.dram_tensor("bfin", [128, 1], dt.float32, kind="ExternalInput").ap()
    y_d = nc.dram_tensor("y", [4, QUARTER], dt.float32, kind="ExternalOutput").ap()

    with tile.TileContext(nc) as tc, ExitStack() as ctx:
        consts = ctx.enter_context(tc.tile_pool(name="consts", bufs=1))
        xpool = ctx.enter_context(tc.tile_pool(name="x", bufs=3))
        h1pool = ctx.enter_context(tc.tile_pool(name="h1", bufs=4))
        h2pool = ctx.enter_context(tc.tile_pool(name="h2", bufs=3))
        stpool = ctx.enter_context(tc.tile_pool(name="stage", bufs=2))
        psA = ctx.enter_context(tc.tile_pool(name="psA", bufs=2, space="PSUM"))
        psB = ctx.enter_context(tc.tile_pool(name="psB", bufs=2, space="PSUM"))
        psE = ctx.enter_context(tc.tile_pool(name="psE", bufs=2, space="PSUM"))
        psG = ctx.enter_context(tc.tile_pool(name="psG", bufs=2, space="PSUM"))

        def cl(dram, shape, dtype):
            t = consts.tile(shape, dtype, tag=dram.tensor.name)
            nc.sync.dma_start(out=t, in_=dram)
            return t

        s_w1a = cl(w1a_d, [128, 128], dt.bfloat16)
        s_w1b = cl(w1b_d, [128, 128], dt.bfloat16)
        s_w2 = cl(w2_d, [128, 64], dt.bfloat16)
        s_w3 = cl(w3_d, [128, 97], dt.bfloat16)
        s_wr = cl(wr_d, [128, 4], dt.bfloat16)
        s_b1r = cl(b1r_d, [128, 1], dt.float32)
        s_b2r = cl(b2r_d, [128, 1], dt.float32)
        s_bf = cl(bf_d, [128, 1], dt.float32)

        w1s = {0: s_w1a, 1: s_w1a, 2: s_w1b, 3: s_w1b}
        # L1 weight slice for pack r, half hh: w1a rows hold packs (0,1),
        # w1b rows hold packs (2,3); cols 0:64 even pack, 64:128 odd pack.
        def w1slice(r, hh):
            w = w1s[r]
            col0 = 64 * (r % 2) + 32 * hh
            return w[32 * r:32 * r + 32, col0:col0 + 32]

        def body():
            for s in range(N_CHUNKS // SGROUP):
                stage = stpool.tile([128, SGROUP * N], dt.float32)
                for jq in range(SGROUP):
                    j = s * SGROUP + jq
                    if j % XGROUP == 0:
                        xbig = xpool.tile([128, XGROUP * N], dt.bfloat16, tag="xbig")
                        nc.sync.dma_start(
                            out=xbig,
                            in_=xt_d[:, (j // XGROUP) * XGROUP * N:
                                     (j // XGROUP + 1) * XGROUP * N])
                    xtile = xbig[:, (j % XGROUP) * N:(j % XGROUP + 1) * N]
                    pA = psA.tile([128, N], dt.float32)
                    pB = psB.tile([128, N], dt.float32)
                    pG = psG.tile([97, N], dt.float32)
                    # L1: 8 concurrent 32x32 tiles + risk on the 4 free
                    # diagonal cells (all 12 use distinct PE sub-arrays)
                    for r in range(4):
                        out_bank = pA if r < 2 else pB
                        ob = 64 * (r % 2)
                        for hh in range(2):
                            nc.tensor.matmul(
                                out_bank[ob + 32 * hh:ob + 32 * hh + 32, :],
                                w1slice(r, hh), xtile[32 * r:32 * r + 32, :],
                                start=True, stop=True,
                                tile_position=(32 * r, ob + 32 * hh))
                        c = POS[r]
                        nc.tensor.matmul(
                            pG[32 * c:32 * c + 1, :],
                            s_wr[32 * r:32 * r + 32, r:r + 1],
                            xtile[32 * r:32 * r + 32, :],
                            start=True, stop=False,
                            tile_position=(32 * r, 32 * c),
                            skip_group_check=True)
                    h1a = h1pool.tile([128, N], dt.bfloat16, tag="h1")
                    nc.scalar.activation(h1a, pA, AF.Relu, bias=s_b1r, scale=1.0)
                    h1b = h1pool.tile([128, N], dt.bfloat16, tag="h1")
                    if jq % 4 == 0:
                        # keep ACT/DVE balanced: 1-in-4 relu1b goes to ACT
                        nc.scalar.activation(h1b, pB, AF.Relu, bias=s_b1r, scale=1.0)
                    else:
                        nc.vector.tensor_scalar(out=h1b, in0=pB, scalar1=s_b1r,
                                                scalar2=0.0, op0=AluOpType.add,
                                                op1=AluOpType.max)
                    pE = psE.tile([128, N], dt.float32)
                    nc.tensor.matmul(pE[0:64, :], s_w2, h1a, start=True, stop=True,
                                     tile_position=(0, 0))
                    nc.tensor.matmul(pE[64:128, :], s_w2, h1b, start=True, stop=True,
                                     tile_position=(0, 64))
                    h2t = h2pool.tile([128, N], dt.bfloat16)
                    nc.vector.tensor_scalar(out=h2t, in0=pE, scalar1=s_b2r,
                                            scalar2=0.0, op0=AluOpType.add,
                                            op1=AluOpType.max)
                    # L3 wide-M: col 32*POS[r] = 0.1*W3 for pack r; zero cols
                    # elsewhere accumulate 0 onto untouched pG rows.
                    nc.tensor.matmul(pG, s_w3, h2t, start=False, stop=True,
                                     skip_group_check=True)
                    nc.scalar.activation(stage[0:97, jq * N:(jq + 1) * N], pG,
                                         AF.Identity, bias=s_bf[0:97], scale=1.0)
                for c in range(4):
                    nc.sync.dma_start(
                        out=y_d[INV[c]:INV[c] + 1,
                                s * SGROUP * N:(s + 1) * SGROUP * N],
                        in_=stage[32 * c:32 * c + 1, :])

        if repeat > 1:
            with tc.For_i(0, repeat, 1):
                body()
        else:
            body()

    nc.compile()
    _BUILD_CACHE[key] = nc
    return nc


def _prep_inputs(x_local, x_all, W1, b1, W2, b2, W3, b3, risk_weights):
    xf = np.empty((B, 28), np.float32)
    xf[:, :4] = x_local
    xf[:, 4:] = x_all[:, 1:7, :].reshape(B, 24)
    xb = xf.astype(BF16)
    X = xb.reshape(N_CORES, 4, QUARTER, 28)

    w1a = np.zeros((128, 128), BF16)
    w1a[0:28, 0:64] = W1
    w1a[32:60, 64:128] = W1
    w1b = np.zeros((128, 128), BF16)
    w1b[64:92, 0:64] = W1
    w1b[96:124, 64:128] = W1
    w2m = np.zeros((128, 64), BF16)
    w2m[0:64, 0:32] = W2
    w2m[64:128, 32:64] = W2
    w3m = np.zeros((128, 97), BF16)
    for r in range(4):
        w3m[32 * r:32 * r + 32, 32 * POS[r]] = 0.1 * W3[:, 0]
    a = np.abs(risk_weights.astype(np.float32))
    e = np.exp(a - a.max())
    wsm = e / e.sum()
    wrm = np.zeros((128, 4), BF16)
    for r in range(4):
        wrm[32 * r:32 * r + 4, r] = -wsm
    b1r = np.tile(np.asarray(b1, np.float32), 2).reshape(128, 1)
    b2r = np.tile(np.asarray(b2, np.float32), 4).reshape(128, 1)
    bfin = np.full((128, 1), 0.3 + 0.1 * float(b3[0]), np.float32)

    consts = dict(w1a=w1a, w1b=w1b, w2=w2m, w3=w3m, wr=wrm,
                  b1r=b1r, b2r=b2r, bfin=bfin)
    in_maps = []
    for c in range(N_CORES):
        xt = np.zeros((4, 32, QUARTER), BF16)
        xt[:, :28, :] = X[c].transpose(0, 2, 1)
        in_maps.append(dict(xt=xt.reshape(128, QUARTER), **consts))
    return in_maps


def run(in_maps, repeat=1):
    from concourse.bass_utils import run_bass_kernel_spmd
    nc = _build(repeat)
    return run_bass_kernel_spmd(nc, in_maps, core_ids=list(range(N_CORES)))


def kernel(x_local, x_all, W1, b1, W2, b2, W3, b3, risk_weights):
    x_local = np.asarray(x_local)
    x_all = np.asarray(x_all)
    in_maps = _prep_inputs(x_local, x_all, np.asarray(W1), np.asarray(b1),
                           np.asarray(W2), np.asarray(b2), np.asarray(W3),
                           np.asarray(b3), np.asarray(risk_weights))
    res = run(in_maps)
    out = np.empty(B, np.float32)
    for c in range(N_CORES):
        out[c * BC:(c + 1) * BC] = np.asarray(res.results[c]["y"],
                                              np.float32).reshape(-1)
    return out
